# revision 11
# baseline (speedup 1.0000x reference)
"""Trainium2 8-core kernel for causal GQA attention (nn_Attention_90967407329949).

Distribution: tensor-parallel over query heads (2 q-heads + their shared
kv-head per core). Each core computes its heads' Q projections for all tokens,
but K/V only for HALF the tokens (core pairs share a kv head; the half is
selected by the per-core `xkv` input slab, so the program stays SPMD-uniform);
a pairwise AllGather exchanges the halves. Attention outputs are AllGathered
per batch (per half-batch for the last batch, to shorten the tail) and each
core computes a 256-column slice of the output projection. The host
concatenates the 8 column slices.

All matmuls run in bf16 (fp32 PSUM accumulation). head_scale is folded into Wo
rows on the host. Softmax skips the running-max (scores are O(1) for this
problem); denominators come from a ones-vector matmul accumulated alongside
the attention*V matmuls.

V tiles are transposed to [token, dh] on the PE (identity-matmul transpose),
NOT via dma_start_transpose: Tile serializes DMA transposes with collectives,
which stalled the pipeline for ~60us in earlier versions.

All collectives are gated by the runtime's ~45us startup barrier on the CC
queue, so attention (which needs the kv exchange) cannot start before ~80us;
the schedule therefore front-loads all projection blocks and runs the
attention phases (ScalarE/exp-bound) interleaved with the remaining q-blocks
and the output projections to keep TensorE dense and HAM-warm.

Layouts (T suffix = transposed, feature dim on SBUF partitions):
  xt   [2048, 4096]   x^T (model dim, b*1024+n tokens), bf16
  xkv  [2048, 2048]   x^T slab this core projects K/V from (per-core half)
  wq   [128, 16, 256] Wq k-tiles: wq[p,t,m] = Wq[t*128+p, c*256+m], bf16
  wk/wv[128, 16, 128] same for this core's kv head, bf16
  wo   [128, 16, 256] (head_scale-folded) Wo k-tiles for this core's col slice
  cost/sint [128, 1024] rotary tables transposed; sint sign-folded
  mask [128, 2048]    4 causal masks for the 4 diagonal offsets
  ident[128, 128]     identity for PE transposes
  out  [256, 4096]    (out @ Wo)^T column slice, bf16 (host upcasts)
"""

import numpy as np
import ml_dtypes

import concourse.bacc as bacc
import concourse.mybir as mybir
import concourse.tile as tile
from concourse.bass_utils import run_bass_kernel_spmd

BF16 = mybir.dt.bfloat16
F32 = mybir.dt.float32

N_CORES = 8
B = 4
N = 1024           # sequence length per batch
NT = B * N         # 4096 tokens
D = 2048           # model dim
DH = 128           # head dim
KT = D // 128      # 16 contraction k-tiles
SCALE = 1.0 / np.sqrt(DH)

_NC_CACHE = {}


def build_nc():
    if "nc" in _NC_CACHE:
        return _NC_CACHE["nc"]
    nc = bacc.Bacc("TRN2", target_bir_lowering=False, debug=False, num_devices=N_CORES)

    xt = nc.dram_tensor("xt", [D, NT], BF16, kind="ExternalInput")
    xkv = nc.dram_tensor("xkv", [D, NT // 2], BF16, kind="ExternalInput")
    wq = nc.dram_tensor("wq", [128, KT, 256], BF16, kind="ExternalInput")
    wk = nc.dram_tensor("wk", [128, KT, 128], BF16, kind="ExternalInput")
    wv = nc.dram_tensor("wv", [128, KT, 128], BF16, kind="ExternalInput")
    wo = nc.dram_tensor("wo", [128, KT, 256], BF16, kind="ExternalInput")
    cost = nc.dram_tensor("cost", [128, N], BF16, kind="ExternalInput")
    sint = nc.dram_tensor("sint", [128, N], BF16, kind="ExternalInput")
    mask = nc.dram_tensor("mask", [128, 2048], BF16, kind="ExternalInput")
    ident = nc.dram_tensor("ident", [128, 128], BF16, kind="ExternalInput")
    out = nc.dram_tensor("out", [256, NT], BF16, kind="ExternalOutput")

    # kv-half exchange: pair AllGather (cores 2h,2h+1). kx j covers this
    # core's xkv blocks 2j,2j+1; gathered rows: [k r0 | v r0 | k r1 | v r1].
    kx_in = [nc.dram_tensor(f"kxi{j}", [256, 1024], BF16) for j in range(2)]
    kx_out = [nc.dram_tensor(f"kxo{j}", [512, 1024], BF16) for j in range(2)]

    # Attention-output AllGathers: merged per batch for b=0..2, per half for
    # b=3 (short tail). Separate DRAM tensors per collective: Tile's DRAM
    # dependency tracking is tensor-granular.
    ag_in = {b: nc.dram_tensor(f"agi{b}", [256, 1024], BF16) for b in range(3)}
    ag_out = {b: nc.dram_tensor(f"ago{b}", [D, 1024], BF16, addr_space="Shared")
              for b in range(3)}
    ag_in3 = [nc.dram_tensor(f"agi3{ib}", [256, 512], BF16) for ib in range(2)]
    ag_out3 = [nc.dram_tensor(f"ago3{ib}", [D, 512], BF16, addr_space="Shared")
               for ib in range(2)]
    ag_out_r = {b: t.rearrange("(t p) n -> p t n", p=128) for b, t in ag_out.items()}
    ag_out3_r = [t.rearrange("(t p) n -> p t n", p=128) for t in ag_out3]

    with tile.TileContext(nc) as tc:
        with (
            tc.tile_pool(name="const", bufs=1) as constp,
            tc.tile_pool(name="persist", bufs=1) as persist,
            tc.tile_pool(name="xtp", bufs=2) as xtp,
            tc.tile_pool(name="xkvp", bufs=2) as xkvp,
            tc.tile_pool(name="qkraw", bufs=2) as qkrawp,
            tc.tile_pool(name="rope", bufs=2) as ropep,
            tc.tile_pool(name="kvtmp", bufs=2) as kvtmpp,
            tc.tile_pool(name="ep", bufs=4) as ep,
            tc.tile_pool(name="etmpp", bufs=2) as etmpp,
            tc.tile_pool(name="attp", bufs=2) as attp,
            tc.tile_pool(name="recipp", bufs=2) as recipp,
            tc.tile_pool(name="rbcp", bufs=2) as rbcp,
            tc.tile_pool(name="gp", bufs=2) as gp,
            tc.tile_pool(name="oobp", bufs=2) as oobp,
            tc.tile_pool(name="psacc", bufs=3, space="PSUM") as psacc,
            tc.tile_pool(name="pss", bufs=2, space="PSUM") as pss,
            tc.tile_pool(name="psu", bufs=2, space="PSUM") as psu,
            tc.tile_pool(name="pssum", bufs=1, space="PSUM") as pssum,
        ):
            # ---- constants ----
            wq_sb = constp.tile([128, KT, 256], BF16)
            wk_sb = constp.tile([128, KT, 128], BF16)
            wv_sb = constp.tile([128, KT, 128], BF16)
            wo_sb = constp.tile([128, KT, 256], BF16)
            cos_sb = constp.tile([128, N], BF16)
            sin_sb = constp.tile([128, N], BF16)
            mask_sb = constp.tile([128, 2048], BF16)
            ones_sb = constp.tile([128, 1], BF16)
            id_sb = constp.tile([128, 128], BF16)
            for c in range(4):  # chunked so the first matmuls start early
                nc.scalar.dma_start(wq_sb[:, c * 4:(c + 1) * 4, :],
                                    wq[:, c * 4:(c + 1) * 4, :])
            nc.scalar.dma_start(wk_sb[:], wk[:])
            nc.scalar.dma_start(wv_sb[:], wv[:])
            nc.scalar.dma_start(id_sb[:], ident[:])
            nc.scalar.dma_start(cos_sb[:], cost[:])
            nc.scalar.dma_start(sin_sb[:], sint[:])
            nc.vector.memset(ones_sb[:], 1.0)

            def late_consts():
                nc.scalar.dma_start(wo_sb[:], wo[:])
                nc.scalar.dma_start(mask_sb[:], mask[:])

            # ---- persistent per-core QKV (RoPE'd, transposed layouts) ----
            q_sb = [persist.tile([128, NT], BF16, name=f"q{h}_sb") for h in range(2)]
            k_sb = persist.tile([128, NT], BF16)
            v_sb = persist.tile([128, NT], BF16)  # 32 [tok,128]x[d,128] tiles

            xt_r = xt.rearrange("(t p) n -> p t n", p=128)
            xkv_r = xkv.rearrange("(t p) n -> p t n", p=128)

            xblks = {}
            xkvblks = {}

            def xblk_load(nb):
                col0 = nb * 512
                xblk = xtp.tile([128, KT, 512], BF16, tag="xblk", name=f"xblk_{nb}")
                ring = nc.sync if nb % 2 == 0 else nc.scalar
                csz = 1 if nb == 0 else 4
                for c0 in range(0, KT, csz):
                    ring.dma_start(xblk[:, c0:c0 + csz, :],
                                   xt_r[:, c0:c0 + csz, col0:col0 + 512])
                xblks[nb] = xblk

            def xkv_load(e):
                col0 = e * 512
                xkb = xkvp.tile([128, KT, 512], BF16, tag="xkv", name=f"xkv_{e}")
                ring = nc.sync if e % 2 == 0 else nc.scalar
                for c0 in range(0, KT, 4):
                    ring.dma_start(xkb[:, c0:c0 + 4, :],
                                   xkv_r[:, c0:c0 + 4, col0:col0 + 512])
                xkvblks[e] = xkb

            def rope_chunk(raw, dst, c0, col0):
                """RoPE 512 positions (table cols c0..c0+512) into dst at col0."""
                rot = ropep.tile([128, 512], BF16, tag="rot")
                nc.sync.dma_start(rot[0:64, :], raw[64:128, :])
                nc.sync.dma_start(rot[64:128, :], raw[0:64, :])
                t1 = ropep.tile([128, 512], BF16, tag="t1")
                nc.vector.tensor_mul(t1[:], raw[:], cos_sb[:, c0:c0 + 512])
                t2 = ropep.tile([128, 512], BF16, tag="t2")
                nc.vector.tensor_mul(t2[:], rot[:], sin_sb[:, c0:c0 + 512])
                nc.vector.tensor_add(dst[:, col0:col0 + 512], t1[:], t2[:])

            def vtrans(src, dst, dcol, tag):
                """PE-transpose 4 [128,128] tiles of src into dst[:, dcol:+512].
                bf16 pass-through psum declared [128,1024] bf16 so the slot
                byte-size matches the psacc tag."""
                vt_ps = psacc.tile([128, 1024], BF16, tag="psacc", name=tag)
                for i in range(4):
                    nc.tensor.matmul(vt_ps[:, i * 128:(i + 1) * 128],
                                     src[:, i * 128:(i + 1) * 128], id_sb[:],
                                     is_transpose=True, skip_group_check=True)
                nc.scalar.activation(dst[:, dcol:dcol + 512], vt_ps[:, 0:512],
                                     mybir.ActivationFunctionType.Copy)

            def qkv_block(nb):
                """Q projection (+K/V from the xkv slab for nb<4) for one
                512-token block, yielding between matmul chunks."""
                col0 = nb * 512
                c0 = (nb % 2) * 512  # rope-table column block
                xblk = xblks[nb]
                if nb == 1:
                    late_consts()

                def accum(dst_ps, w_sb, msl, blk):
                    for k0 in range(0, KT, 4):
                        for kt in range(k0, k0 + 4):
                            nc.tensor.matmul(
                                dst_ps, w_sb[:, kt, msl], blk[:, kt, :],
                                start=(kt == 0), stop=(kt == KT - 1))
                        yield

                # Q (2 head-tiles)
                for m in range(2):
                    raw = qkrawp.tile([128, 512], BF16, tag=f"qraw{m}",
                                      name=f"qraw{m}_{nb}")
                    q_ps = psacc.tile([128, 512], F32, tag="psacc",
                                      name=f"q_ps_{nb}_{m}")
                    yield from accum(q_ps[:], wq_sb,
                                     slice(m * 128, (m + 1) * 128), xblk)
                    nc.scalar.activation(raw[:], q_ps[:],
                                         mybir.ActivationFunctionType.Copy)
                    yield
                    rope_chunk(raw, q_sb[m], c0, col0)

                if nb >= 4:
                    return
                # K/V for this core's kv-token half, block nb of xkv.
                xkb = xkvblks[nb]
                kraw = qkrawp.tile([128, 512], BF16, tag="kraw", name=f"kraw_{nb}")
                k_ps = psacc.tile([128, 512], F32, tag="psacc", name=f"k_ps_{nb}")
                yield from accum(k_ps[:], wk_sb, slice(0, 128), xkb)
                nc.scalar.activation(kraw[:], k_ps[:],
                                     mybir.ActivationFunctionType.Copy)
                yield
                v_ps = psacc.tile([128, 512], F32, tag="psacc", name=f"v_ps_{nb}")
                yield from accum(v_ps[:], wv_sb, slice(0, 128), xkb)
                vraw = ropep.tile([128, 512], BF16, tag="vraw")
                nc.scalar.activation(vraw[:], v_ps[:],
                                     mybir.ActivationFunctionType.Copy)
                yield
                ktmp = kvtmpp.tile([128, 512], BF16, tag="ktmp", name=f"ktmp_{nb}")
                rope_chunk(kraw, ktmp, c0, 0)
                nc.sync.dma_start(
                    kx_in[nb // 2][0:128, (nb % 2) * 512:(nb % 2 + 1) * 512],
                    ktmp[:])
                vtmp = kvtmpp.tile([128, 512], BF16, tag="vtmp", name=f"vtmp_{nb}")
                vtrans(vraw, vtmp, 0, f"vt_ps_{nb}")
                nc.sync.dma_start(
                    kx_in[nb // 2][128:256, (nb % 2) * 512:(nb % 2 + 1) * 512],
                    vtmp[:])
                yield

            def kx_exchange(j):
                nc.gpsimd.collective_compute(
                    "AllGather",
                    mybir.AluOpType.bypass,
                    replica_groups=[[2 * h, 2 * h + 1] for h in range(4)],
                    ins=[kx_in[j][:].opt()],
                    outs=[kx_out[j][:].opt()],
                )
                # assemble k_sb/v_sb in global token order (rank0 of the pair
                # owns tokens 0..2047, rank1 2048..4095 -- identical on both).
                # On gpsimd: these wait for the collective, and everything
                # behind them on that queue (broadcasts, ag_in writes) comes
                # later anyway -- no head-of-line blocking.
                nc.gpsimd.dma_start(k_sb[:, j * 1024:(j + 1) * 1024],
                                    kx_out[j][0:128, :])
                nc.gpsimd.dma_start(v_sb[:, j * 1024:(j + 1) * 1024],
                                    kx_out[j][128:256, :])
                nc.gpsimd.dma_start(k_sb[:, 2048 + j * 1024:2048 + (j + 1) * 1024],
                                    kx_out[j][256:384, :])
                nc.gpsimd.dma_start(v_sb[:, 2048 + j * 1024:2048 + (j + 1) * 1024],
                                    kx_out[j][384:512, :])

            def att_ib(b, ib):
                """Attention for (batch, 512-token i-half), both heads,
                yielding between j-tile units."""
                icol = b * N + ib * 512
                cnt = 4 * ib + 4
                for h in range(2):
                    qh = q_sb[h]
                    att = attp.tile([128, 512], BF16, tag="att",
                                    name=f"att_{b}_{ib}_{h}")
                    u_ps = psu.tile([128, 512], F32, tag="psu",
                                    name=f"u_ps_{b}_{ib}_{h}")
                    sum_ps = pssum.tile([1, 512], F32, tag="pssum",
                                        name=f"sum_ps_{b}_{ib}_{h}")

                    def c_lo(jt):
                        # diagonal tile at offset r: columns < 128*r are
                        # causally invalid for every row -- skip them in
                        # every consumer (exact: those (j,i) pairs are
                        # fully masked, and sum/u accumulation over the
                        # remaining tiles covers the kept columns).
                        r = jt - 4 * ib
                        return 128 * r if r > 0 else 0

                    def s_mm(jt):
                        s_ps = pss.tile([128, 512], F32, tag="pss",
                                        name=f"s_ps_{b}_{ib}_{h}_{jt}")
                        jcol = b * N + jt * 128
                        c0 = c_lo(jt)
                        nc.tensor.matmul(
                            s_ps[:, c0:512], k_sb[:, jcol:jcol + 128],
                            qh[:, icol + c0:icol + 512],
                            start=True, stop=True)
                        return s_ps

                    def e_of(jt, s_ps):
                        r = jt - 4 * ib
                        c0 = c_lo(jt)
                        e = ep.tile([128, 512], BF16, tag="e",
                                    name=f"e_{b}_{ib}_{h}_{jt}")
                        if r >= 0:  # diagonal tile: mask after exp
                            etmp = etmpp.tile([128, 512], BF16, tag="etmp")
                            nc.scalar.activation(
                                etmp[:, c0:512], s_ps[:, c0:512],
                                mybir.ActivationFunctionType.Exp, scale=SCALE)
                            nc.vector.tensor_mul(
                                e[:, c0:512], etmp[:, c0:512],
                                mask_sb[:, r * 512 + c0:(r + 1) * 512])
                        else:
                            nc.scalar.activation(
                                e[:], s_ps[:],
                                mybir.ActivationFunctionType.Exp, scale=SCALE)
                        return e

                    s_tiles = {0: s_mm(0), 1: s_mm(1)}
                    for jt in range(cnt):
                        e = e_of(jt, s_tiles.pop(jt))
                        if jt + 2 < cnt:
                            s_tiles[jt + 2] = s_mm(jt + 2)
                        tt = b * 8 + jt
                        c0 = c_lo(jt)
                        nc.tensor.matmul(
                            u_ps[:, c0:512],
                            v_sb[:, tt * 128:(tt + 1) * 128], e[:, c0:512],
                            start=(jt == 0), stop=(jt == cnt - 1),
                            skip_group_check=True)
                        nc.tensor.matmul(
                            sum_ps[:, c0:512], ones_sb[:], e[:, c0:512],
                            start=(jt == 0), stop=(jt == cnt - 1),
                            skip_group_check=True)
                        yield
                    recip = recipp.tile([1, 512], F32, tag="recip")
                    nc.vector.reciprocal_approx_fast(out=recip[:], in_=sum_ps[:])
                    rbc = rbcp.tile([128, 512], F32, tag="rbc")
                    nc.gpsimd.partition_broadcast(rbc[:], recip[:])
                    nc.vector.tensor_mul(att[:], u_ps[:], rbc[:])
                    if b < 3:
                        nc.gpsimd.dma_start(
                            ag_in[b][h * 128:(h + 1) * 128,
                                     ib * 512:(ib + 1) * 512], att[:])
                    else:
                        nc.gpsimd.dma_start(
                            ag_in3[ib][h * 128:(h + 1) * 128, :], att[:])
                    yield

            def allgather_b(b):
                nc.gpsimd.collective_compute(
                    "AllGather",
                    mybir.AluOpType.bypass,
                    replica_groups=[list(range(N_CORES))],
                    ins=[ag_in[b][:].opt()],
                    outs=[ag_out[b][:].opt()],
                )

            def allgather3(ib):
                nc.gpsimd.collective_compute(
                    "AllGather",
                    mybir.AluOpType.bypass,
                    replica_groups=[list(range(N_CORES))],
                    ins=[ag_in3[ib][:].opt()],
                    outs=[ag_out3[ib][:].opt()],
                )

            g_tiles = {}

            def g_prefetch(b, ib):
                """Load one gathered [2048, 512] slab for the out projection."""
                g_tiles[(b, ib)] = gp.tile([128, KT, 512], BF16, tag="g",
                                           name=f"g_{b}_{ib}")
                ring = nc.sync if ib == 0 else nc.scalar
                if b < 3:
                    src = ag_out_r[b][:, :, ib * 512:(ib + 1) * 512]
                else:
                    src = ag_out3_r[ib][:]
                for c0 in range(0, KT, 8):
                    ring.dma_start(g_tiles[(b, ib)][:, c0:c0 + 8, :],
                                   src[:, c0:c0 + 8, :])

            def oproj_ib(b, ib):
                g = g_tiles.pop((b, ib))
                for m in range(2):
                    o_ps = psacc.tile([128, 512], F32, tag="psacc",
                                      name=f"o_ps_{b}_{ib}_{m}")
                    for k0 in range(0, KT, 4):
                        for kt in range(k0, k0 + 4):
                            nc.tensor.matmul(
                                o_ps[:], wo_sb[:, kt, m * 128:(m + 1) * 128],
                                g[:, kt, :], start=(kt == 0),
                                stop=(kt == KT - 1))
                        yield
                    osb = oobp.tile([128, 512], BF16, tag="osb",
                                    name=f"osb_{b}_{ib}_{m}")
                    nc.vector.tensor_copy(osb[:], o_ps[:])
                    nc.gpsimd.dma_start(
                        out[m * 128:(m + 1) * 128,
                            b * N + ib * 512:b * N + (ib + 1) * 512], osb[:])
                    yield

            def drain(gen):
                for _ in gen:
                    pass

            def chain(*gens):
                for g in gens:
                    yield from g

            def interleave(gen_a, gen_b, na=1, nb=1, prime_b=0):
                """Alternate generators, na steps of gen_a per nb of gen_b,
                optionally priming gen_b first."""
                for _ in range(prime_b):
                    try:
                        next(gen_b)
                    except StopIteration:
                        break
                alive = [gen_a, gen_b]
                while alive:
                    for g in list(alive):
                        steps = na if g is gen_a else nb
                        for _ in range(steps):
                            try:
                                next(g)
                            except StopIteration:
                                if g in alive:
                                    alive.remove(g)
                                break

            # ---- schedule ----
            # Front-load projections (collectives are barrier-gated anyway);
            # run attention (exp-bound) interleaved with remaining q-blocks
            # and output projections; keep the last batch's collectives small.
            xblk_load(0)
            xkv_load(0)
            xblk_load(1)
            xkv_load(1)
            drain(qkv_block(0))
            xblk_load(2)
            xkv_load(2)
            drain(qkv_block(1))
            kx_exchange(0)
            xblk_load(3)
            xkv_load(3)
            drain(qkv_block(2))
            xblk_load(4)
            drain(qkv_block(3))
            kx_exchange(1)
            xblk_load(5)
            interleave(chain(att_ib(0, 0), att_ib(0, 1)),
                       chain(qkv_block(4), qkv_block(5)), na=1, nb=1, prime_b=4)
            allgather_b(0)
            xblk_load(6)
            xblk_load(7)
            interleave(chain(att_ib(1, 0), att_ib(1, 1)),
                       chain(qkv_block(6), qkv_block(7)), na=1, nb=1, prime_b=2)
            allgather_b(1)
            g_prefetch(0, 0)
            g_prefetch(0, 1)
            interleave(chain(att_ib(2, 0), att_ib(2, 1)),
                       chain(oproj_ib(0, 0), oproj_ib(0, 1)),
                       na=1, nb=1, prime_b=2)
            allgather_b(2)
            g_prefetch(1, 0)
            g_prefetch(1, 1)
            interleave(att_ib(3, 0), chain(oproj_ib(1, 0), oproj_ib(1, 1)),
                       na=1, nb=2, prime_b=2)
            allgather3(0)
            g_prefetch(2, 0)
            g_prefetch(2, 1)
            interleave(att_ib(3, 1), chain(oproj_ib(2, 0), oproj_ib(2, 1)),
                       na=1, nb=2, prime_b=2)
            allgather3(1)
            g_prefetch(3, 0)
            drain(oproj_ib(3, 0))
            g_prefetch(3, 1)
            drain(oproj_ib(3, 1))

    nc.compile()
    _NC_CACHE["nc"] = nc
    return nc


def _host_prep(x, Wq, Wk, Wv, Wo, head_scale):
    bf = ml_dtypes.bfloat16
    xt = np.ascontiguousarray(x.reshape(NT, D).T).astype(bf)

    hs = np.asarray(head_scale).reshape(16)
    wo_s = (np.asarray(Wo) * np.repeat(hs, DH)[:, None]).astype(np.float32)

    def ktile(w):  # [2048, M] -> [128, 16, M]
        m = w.shape[1]
        return np.ascontiguousarray(
            w.reshape(KT, 128, m).transpose(1, 0, 2)).astype(bf)

    inv_freq = (1.0 / (10000.0 ** (np.arange(0, DH, 2, dtype=np.float64) / DH)))
    freqs = np.arange(N, dtype=np.float64)[:, None] * inv_freq[None, :]  # [N, 64]
    emb = np.concatenate([freqs, freqs], axis=-1)  # [N, 128]
    cosT = np.ascontiguousarray(np.cos(emb).T).astype(bf)  # [128, N]
    sinT = np.sin(emb).T  # [128, N]
    sign = np.where(np.arange(DH) < 64, -1.0, 1.0)[:, None]
    sinT = np.ascontiguousarray(sinT * sign).astype(bf)

    # 4 diagonal masks r=0..3: valid (c >= p + 128*r)
    p = np.arange(128)[:, None]
    c = np.arange(512)[None, :]
    masks = [(c >= p + 128 * r).astype(np.float32) for r in range(4)]
    mask = np.concatenate(masks, axis=1).astype(bf)  # [128, 2048]

    idm = np.eye(128, dtype=np.float32).astype(bf)

    in_maps = []
    for core in range(N_CORES):
        kv = core // 2
        half = core % 2
        in_maps.append({
            "xt": xt,
            "xkv": np.ascontiguousarray(xt[:, half * 2048:(half + 1) * 2048]),
            "wq": ktile(np.asarray(Wq)[:, core * 256:(core + 1) * 256]),
            "wk": ktile(np.asarray(Wk)[:, kv * 128:(kv + 1) * 128]),
            "wv": ktile(np.asarray(Wv)[:, kv * 128:(kv + 1) * 128]),
            "wo": ktile(wo_s[:, core * 256:(core + 1) * 256]),
            "cost": cosT,
            "sint": sinT,
            "mask": mask,
            "ident": idm,
        })
    return in_maps


def kernel(x, Wq, Wk, Wv, Wo, head_scale, _run_kwargs=None):
    nc = build_nc()
    in_maps = _host_prep(x, Wq, Wk, Wv, Wo, head_scale)
    res = run_bass_kernel_spmd(
        nc, in_maps, core_ids=list(range(N_CORES)), **(_run_kwargs or {})
    )
    outT = np.concatenate(
        [res.results[c]["out"].astype(np.float32) for c in range(N_CORES)], axis=0)
    full = np.ascontiguousarray(outT.T).reshape(B, N, D)
    if _run_kwargs:
        kernel.last_results = res
    return full


# revision 15
# speedup vs baseline: 1.0311x; 1.0311x over previous
"""Trainium2 8-core kernel for causal GQA attention (nn_Attention_90967407329949).

Distribution: tensor-parallel over query heads (2 q-heads + their shared
kv-head per core). Each core computes its heads' Q projections for all tokens,
but K/V only for HALF the tokens (core pairs share a kv head; the half is
selected by the per-core `xkv` input slab, so the program stays SPMD-uniform);
a pairwise AllGather exchanges the halves. Attention outputs are AllGathered
per batch (per half-batch for the last batch, to shorten the tail) and each
core computes a 256-column slice of the output projection. The host
concatenates the 8 column slices.

All matmuls run in bf16 (fp32 PSUM accumulation). head_scale is folded into Wo
rows on the host. Softmax skips the running-max (scores are O(1) for this
problem); denominators come from a ones-vector matmul accumulated alongside
the attention*V matmuls.

V tiles are transposed to [token, dh] on the PE (identity-matmul transpose),
NOT via dma_start_transpose: Tile serializes DMA transposes with collectives,
which stalled the pipeline for ~60us in earlier versions.

All collectives are gated by the runtime's ~45us startup barrier on the CC
queue, so attention (which needs the kv exchange) cannot start before ~80us;
the schedule therefore front-loads all projection blocks and runs the
attention phases (ScalarE/exp-bound) interleaved with the remaining q-blocks
and the output projections to keep TensorE dense and HAM-warm.

Layouts (T suffix = transposed, feature dim on SBUF partitions):
  xt   [2048, 4096]   x^T (model dim, b*1024+n tokens), bf16
  xkv  [2048, 2048]   x^T slab this core projects K/V from (per-core half)
  wq   [128, 16, 256] Wq k-tiles: wq[p,t,m] = Wq[t*128+p, c*256+m], bf16
  wk/wv[128, 16, 128] same for this core's kv head, bf16
  wo   [128, 16, 256] (head_scale-folded) Wo k-tiles for this core's col slice
  cost/sint [128, 1024] rotary tables transposed; sint sign-folded
  mask [128, 2048]    4 causal masks for the 4 diagonal offsets
  ident[128, 128]     identity for PE transposes
  out  [256, 4096]    (out @ Wo)^T column slice, bf16 (host upcasts)
"""

import numpy as np
import ml_dtypes

import concourse.bacc as bacc
import concourse.mybir as mybir
import concourse.tile as tile
from concourse.bass_utils import run_bass_kernel_spmd

BF16 = mybir.dt.bfloat16
F32 = mybir.dt.float32

N_CORES = 8
B = 4
N = 1024           # sequence length per batch
NT = B * N         # 4096 tokens
D = 2048           # model dim
DH = 128           # head dim
KT = D // 128      # 16 contraction k-tiles
SCALE = 1.0 / np.sqrt(DH)

_NC_CACHE = {}


def build_nc():
    if "nc" in _NC_CACHE:
        return _NC_CACHE["nc"]
    nc = bacc.Bacc("TRN2", target_bir_lowering=False, debug=False, num_devices=N_CORES)

    xt = nc.dram_tensor("xt", [D, NT], BF16, kind="ExternalInput")
    xkv = nc.dram_tensor("xkv", [D, NT // 2], BF16, kind="ExternalInput")
    wq = nc.dram_tensor("wq", [128, KT, 256], BF16, kind="ExternalInput")
    wk = nc.dram_tensor("wk", [128, KT, 128], BF16, kind="ExternalInput")
    wv = nc.dram_tensor("wv", [128, KT, 128], BF16, kind="ExternalInput")
    wo = nc.dram_tensor("wo", [128, KT, 256], BF16, kind="ExternalInput")
    cost = nc.dram_tensor("cost", [128, N], BF16, kind="ExternalInput")
    sint = nc.dram_tensor("sint", [128, N], BF16, kind="ExternalInput")
    mask = nc.dram_tensor("mask", [128, 2048], BF16, kind="ExternalInput")
    ident = nc.dram_tensor("ident", [128, 128], BF16, kind="ExternalInput")
    out = nc.dram_tensor("out", [256, NT], BF16, kind="ExternalOutput")

    # kv-half exchange: pair AllGather (cores 2h,2h+1). kx j covers this
    # core's xkv blocks 2j,2j+1; gathered rows: [k r0 | v r0 | k r1 | v r1].
    kx_in = [nc.dram_tensor(f"kxi{j}", [256, 1024], BF16) for j in range(2)]
    kx_out = [nc.dram_tensor(f"kxo{j}", [512, 1024], BF16) for j in range(2)]

    # Attention-output AllGathers: merged per batch for b=0..2, per half for
    # b=3 (short tail). Separate DRAM tensors per collective: Tile's DRAM
    # dependency tracking is tensor-granular.
    ag_in = {b: nc.dram_tensor(f"agi{b}", [256, 1024], BF16) for b in range(3)}
    ag_out = {b: nc.dram_tensor(f"ago{b}", [D, 1024], BF16, addr_space="Shared")
              for b in range(3)}
    ag_in3 = [nc.dram_tensor(f"agi3{ib}", [256, 512], BF16) for ib in range(2)]
    ag_out3 = [nc.dram_tensor(f"ago3{ib}", [D, 512], BF16, addr_space="Shared")
               for ib in range(2)]
    ag_out_r = {b: t.rearrange("(t p) n -> p t n", p=128) for b, t in ag_out.items()}
    ag_out3_r = [t.rearrange("(t p) n -> p t n", p=128) for t in ag_out3]

    with tile.TileContext(nc) as tc:
        with (
            tc.tile_pool(name="const", bufs=1) as constp,
            tc.tile_pool(name="persist", bufs=1) as persist,
            tc.tile_pool(name="xtp", bufs=2) as xtp,
            tc.tile_pool(name="xkvp", bufs=2) as xkvp,
            tc.tile_pool(name="qkraw", bufs=2) as qkrawp,
            tc.tile_pool(name="rope", bufs=2) as ropep,
            tc.tile_pool(name="kvtmp", bufs=2) as kvtmpp,
            tc.tile_pool(name="ep", bufs=4) as ep,
            tc.tile_pool(name="etmpp", bufs=2) as etmpp,
            tc.tile_pool(name="attp", bufs=2) as attp,
            tc.tile_pool(name="recipp", bufs=2) as recipp,
            tc.tile_pool(name="rbcp", bufs=2) as rbcp,
            tc.tile_pool(name="gp", bufs=2) as gp,
            tc.tile_pool(name="oobp", bufs=2) as oobp,
            tc.tile_pool(name="psacc", bufs=3, space="PSUM") as psacc,
            tc.tile_pool(name="pss", bufs=2, space="PSUM") as pss,
            tc.tile_pool(name="psu", bufs=2, space="PSUM") as psu,
            tc.tile_pool(name="pssum", bufs=1, space="PSUM") as pssum,
        ):
            # ---- constants ----
            wq_sb = constp.tile([128, KT, 256], BF16)
            wk_sb = constp.tile([128, KT, 128], BF16)
            wv_sb = constp.tile([128, KT, 128], BF16)
            wo_sb = constp.tile([128, KT, 256], BF16)
            cos_sb = constp.tile([128, N], BF16)
            sin_sb = constp.tile([128, N], BF16)
            mask_sb = constp.tile([128, 2048], BF16)
            ones_sb = constp.tile([128, 1], BF16)
            id_sb = constp.tile([128, 128], BF16)
            nc.scalar.dma_start(wq_sb[:, 0:4, :], wq[:, 0:4, :])
            nc.vector.memset(ones_sb[:], 1.0)

            def early_consts():  # emitted after block-0 loads
                for c in range(1, 4):
                    nc.scalar.dma_start(wq_sb[:, c * 4:(c + 1) * 4, :],
                                        wq[:, c * 4:(c + 1) * 4, :])
                nc.scalar.dma_start(wk_sb[:], wk[:])
                nc.scalar.dma_start(wv_sb[:], wv[:])
                nc.scalar.dma_start(id_sb[:], ident[:])
                nc.scalar.dma_start(cos_sb[:], cost[:])
                nc.scalar.dma_start(sin_sb[:], sint[:])

            def late_consts():
                nc.scalar.dma_start(wo_sb[:], wo[:])
                nc.scalar.dma_start(mask_sb[:], mask[:])

            # ---- persistent per-core QKV (RoPE'd, transposed layouts) ----
            q_sb = [persist.tile([128, NT], BF16, name=f"q{h}_sb") for h in range(2)]
            k_sb = persist.tile([128, NT], BF16)
            v_sb = persist.tile([128, NT], BF16)  # 32 [tok,128]x[d,128] tiles

            xt_r = xt.rearrange("(t p) n -> p t n", p=128)
            xkv_r = xkv.rearrange("(t p) n -> p t n", p=128)

            xblks = {}
            xkvblks = {}

            def xblk_load(nb):
                col0 = nb * 512
                xblk = xtp.tile([128, KT, 512], BF16, tag="xblk", name=f"xblk_{nb}")
                csz = 4 if nb == 0 else 8
                for c0 in range(0, KT, csz):
                    nc.scalar.dma_start(xblk[:, c0:c0 + csz, :],
                                        xt_r[:, c0:c0 + csz, col0:col0 + 512])
                xblks[nb] = xblk

            def xkv_load(e):
                col0 = e * 512
                xkb = xkvp.tile([128, KT, 512], BF16, tag="xkv", name=f"xkv_{e}")
                for c0 in range(0, KT, 8):
                    nc.scalar.dma_start(xkb[:, c0:c0 + 8, :],
                                        xkv_r[:, c0:c0 + 8, col0:col0 + 512])
                xkvblks[e] = xkb

            def rope_chunk(raw, dst, c0, col0):
                """RoPE 512 positions (table cols c0..c0+512) into dst at col0."""
                rot = ropep.tile([128, 512], BF16, tag="rot")
                nc.sync.dma_start(rot[0:64, :], raw[64:128, :])
                nc.sync.dma_start(rot[64:128, :], raw[0:64, :])
                t1 = ropep.tile([128, 512], BF16, tag="t1")
                nc.vector.tensor_mul(t1[:], raw[:], cos_sb[:, c0:c0 + 512])
                t2 = ropep.tile([128, 512], BF16, tag="t2")
                nc.vector.tensor_mul(t2[:], rot[:], sin_sb[:, c0:c0 + 512])
                nc.vector.tensor_add(dst[:, col0:col0 + 512], t1[:], t2[:])

            def vtrans(src, dst, dcol, tag):
                """PE-transpose 4 [128,128] tiles of src into dst[:, dcol:+512].
                bf16 pass-through psum declared [128,1024] bf16 so the slot
                byte-size matches the psacc tag."""
                vt_ps = psacc.tile([128, 1024], BF16, tag="psacc", name=tag)
                for i in range(4):
                    nc.tensor.matmul(vt_ps[:, i * 128:(i + 1) * 128],
                                     src[:, i * 128:(i + 1) * 128], id_sb[:],
                                     is_transpose=True, skip_group_check=True)
                nc.scalar.activation(dst[:, dcol:dcol + 512], vt_ps[:, 0:512],
                                     mybir.ActivationFunctionType.Copy)

            def qkv_block(nb):
                """Q projection (+K/V from the xkv slab for nb<4) for one
                512-token block, yielding between matmul chunks."""
                col0 = nb * 512
                c0 = (nb % 2) * 512  # rope-table column block
                xblk = xblks[nb]
                if nb == 1:
                    late_consts()

                def accum(dst_ps, w_sb, msl, blk):
                    for k0 in range(0, KT, 4):
                        for kt in range(k0, k0 + 4):
                            nc.tensor.matmul(
                                dst_ps, w_sb[:, kt, msl], blk[:, kt, :],
                                start=(kt == 0), stop=(kt == KT - 1))
                        yield

                # Q (2 head-tiles)
                for m in range(2):
                    raw = qkrawp.tile([128, 512], BF16, tag=f"qraw{m}",
                                      name=f"qraw{m}_{nb}")
                    q_ps = psacc.tile([128, 512], F32, tag="psacc",
                                      name=f"q_ps_{nb}_{m}")
                    yield from accum(q_ps[:], wq_sb,
                                     slice(m * 128, (m + 1) * 128), xblk)
                    nc.scalar.activation(raw[:], q_ps[:],
                                         mybir.ActivationFunctionType.Copy)
                    yield
                    rope_chunk(raw, q_sb[m], c0, col0)

                if nb >= 4:
                    return
                # K/V for this core's kv-token half, block nb of xkv.
                xkb = xkvblks[nb]
                kraw = qkrawp.tile([128, 512], BF16, tag="kraw", name=f"kraw_{nb}")
                k_ps = psacc.tile([128, 512], F32, tag="psacc", name=f"k_ps_{nb}")
                yield from accum(k_ps[:], wk_sb, slice(0, 128), xkb)
                nc.scalar.activation(kraw[:], k_ps[:],
                                     mybir.ActivationFunctionType.Copy)
                yield
                v_ps = psacc.tile([128, 512], F32, tag="psacc", name=f"v_ps_{nb}")
                yield from accum(v_ps[:], wv_sb, slice(0, 128), xkb)
                vraw = ropep.tile([128, 512], BF16, tag="vraw")
                nc.scalar.activation(vraw[:], v_ps[:],
                                     mybir.ActivationFunctionType.Copy)
                yield
                ktmp = kvtmpp.tile([128, 512], BF16, tag="ktmp", name=f"ktmp_{nb}")
                rope_chunk(kraw, ktmp, c0, 0)
                nc.sync.dma_start(
                    kx_in[nb // 2][0:128, (nb % 2) * 512:(nb % 2 + 1) * 512],
                    ktmp[:])
                vtmp = kvtmpp.tile([128, 512], BF16, tag="vtmp", name=f"vtmp_{nb}")
                vtrans(vraw, vtmp, 0, f"vt_ps_{nb}")
                nc.sync.dma_start(
                    kx_in[nb // 2][128:256, (nb % 2) * 512:(nb % 2 + 1) * 512],
                    vtmp[:])
                yield

            def kx_exchange(j):
                nc.gpsimd.collective_compute(
                    "AllGather",
                    mybir.AluOpType.bypass,
                    replica_groups=[[2 * h, 2 * h + 1] for h in range(4)],
                    ins=[kx_in[j][:].opt()],
                    outs=[kx_out[j][:].opt()],
                )
                # assemble k_sb/v_sb in global token order (rank0 of the pair
                # owns tokens 0..2047, rank1 2048..4095 -- identical on both).
                # On gpsimd: these wait for the collective, and everything
                # behind them on that queue (broadcasts, ag_in writes) comes
                # later anyway -- no head-of-line blocking.
                nc.gpsimd.dma_start(k_sb[:, j * 1024:(j + 1) * 1024],
                                    kx_out[j][0:128, :])
                nc.gpsimd.dma_start(v_sb[:, j * 1024:(j + 1) * 1024],
                                    kx_out[j][128:256, :])
                nc.gpsimd.dma_start(k_sb[:, 2048 + j * 1024:2048 + (j + 1) * 1024],
                                    kx_out[j][256:384, :])
                nc.gpsimd.dma_start(v_sb[:, 2048 + j * 1024:2048 + (j + 1) * 1024],
                                    kx_out[j][384:512, :])

            def att_ib(b, ib):
                """Attention for (batch, 512-token i-half), both heads,
                yielding between j-tile units."""
                icol = b * N + ib * 512
                cnt = 4 * ib + 4
                for h in range(2):
                    qh = q_sb[h]
                    att = attp.tile([128, 512], BF16, tag="att",
                                    name=f"att_{b}_{ib}_{h}")
                    u_ps = psu.tile([128, 512], F32, tag="psu",
                                    name=f"u_ps_{b}_{ib}_{h}")
                    sum_ps = pssum.tile([1, 512], F32, tag="pssum",
                                        name=f"sum_ps_{b}_{ib}_{h}")

                    def c_lo(jt):
                        # diagonal tile at offset r: columns < 128*r are
                        # causally invalid for every row -- skip them in
                        # every consumer (exact: those (j,i) pairs are
                        # fully masked, and sum/u accumulation over the
                        # remaining tiles covers the kept columns).
                        r = jt - 4 * ib
                        return 128 * r if r > 0 else 0

                    def s_mm(jt):
                        s_ps = pss.tile([128, 512], F32, tag="pss",
                                        name=f"s_ps_{b}_{ib}_{h}_{jt}")
                        jcol = b * N + jt * 128
                        c0 = c_lo(jt)
                        nc.tensor.matmul(
                            s_ps[:, c0:512], k_sb[:, jcol:jcol + 128],
                            qh[:, icol + c0:icol + 512],
                            start=True, stop=True)
                        return s_ps

                    def e_of(jt, s_ps):
                        r = jt - 4 * ib
                        c0 = c_lo(jt)
                        e = ep.tile([128, 512], BF16, tag="e",
                                    name=f"e_{b}_{ib}_{h}_{jt}")
                        if r >= 0:  # diagonal tile: mask after exp
                            etmp = etmpp.tile([128, 512], BF16, tag="etmp")
                            nc.scalar.activation(
                                etmp[:, c0:512], s_ps[:, c0:512],
                                mybir.ActivationFunctionType.Exp, scale=SCALE)
                            nc.vector.tensor_mul(
                                e[:, c0:512], etmp[:, c0:512],
                                mask_sb[:, r * 512 + c0:(r + 1) * 512])
                        else:
                            nc.scalar.activation(
                                e[:], s_ps[:],
                                mybir.ActivationFunctionType.Exp, scale=SCALE)
                        return e

                    s_tiles = {0: s_mm(0), 1: s_mm(1)}
                    for jt in range(cnt):
                        e = e_of(jt, s_tiles.pop(jt))
                        if jt + 2 < cnt:
                            s_tiles[jt + 2] = s_mm(jt + 2)
                        tt = b * 8 + jt
                        c0 = c_lo(jt)
                        nc.tensor.matmul(
                            u_ps[:, c0:512],
                            v_sb[:, tt * 128:(tt + 1) * 128], e[:, c0:512],
                            start=(jt == 0), stop=(jt == cnt - 1),
                            skip_group_check=True)
                        nc.tensor.matmul(
                            sum_ps[:, c0:512], ones_sb[:], e[:, c0:512],
                            start=(jt == 0), stop=(jt == cnt - 1),
                            skip_group_check=True)
                        yield
                    recip = recipp.tile([1, 512], F32, tag="recip")
                    nc.vector.reciprocal_approx_fast(out=recip[:], in_=sum_ps[:])
                    rbc = rbcp.tile([128, 512], F32, tag="rbc")
                    nc.gpsimd.partition_broadcast(rbc[:], recip[:])
                    nc.vector.tensor_mul(att[:], u_ps[:], rbc[:])
                    if b < 3:
                        nc.gpsimd.dma_start(
                            ag_in[b][h * 128:(h + 1) * 128,
                                     ib * 512:(ib + 1) * 512], att[:])
                    else:
                        nc.gpsimd.dma_start(
                            ag_in3[ib][h * 128:(h + 1) * 128, :], att[:])
                    yield

            def allgather_b(b):
                nc.gpsimd.collective_compute(
                    "AllGather",
                    mybir.AluOpType.bypass,
                    replica_groups=[list(range(N_CORES))],
                    ins=[ag_in[b][:].opt()],
                    outs=[ag_out[b][:].opt()],
                )

            def allgather3(ib):
                nc.gpsimd.collective_compute(
                    "AllGather",
                    mybir.AluOpType.bypass,
                    replica_groups=[list(range(N_CORES))],
                    ins=[ag_in3[ib][:].opt()],
                    outs=[ag_out3[ib][:].opt()],
                )

            g_tiles = {}

            def g_prefetch(b, ib):
                """Load one gathered [2048, 512] slab for the out projection."""
                g_tiles[(b, ib)] = gp.tile([128, KT, 512], BF16, tag="g",
                                           name=f"g_{b}_{ib}")
                if b < 3:
                    src = ag_out_r[b][:, :, ib * 512:(ib + 1) * 512]
                else:
                    src = ag_out3_r[ib][:]
                for c0 in range(0, KT, 8):
                    nc.gpsimd.dma_start(g_tiles[(b, ib)][:, c0:c0 + 8, :],
                                        src[:, c0:c0 + 8, :])

            def oproj_ib(b, ib):
                g = g_tiles.pop((b, ib))
                for m in range(2):
                    o_ps = psacc.tile([128, 512], F32, tag="psacc",
                                      name=f"o_ps_{b}_{ib}_{m}")
                    for k0 in range(0, KT, 4):
                        for kt in range(k0, k0 + 4):
                            nc.tensor.matmul(
                                o_ps[:], wo_sb[:, kt, m * 128:(m + 1) * 128],
                                g[:, kt, :], start=(kt == 0),
                                stop=(kt == KT - 1))
                        yield
                    osb = oobp.tile([128, 512], BF16, tag="osb",
                                    name=f"osb_{b}_{ib}_{m}")
                    nc.vector.tensor_copy(osb[:], o_ps[:])
                    nc.gpsimd.dma_start(
                        out[m * 128:(m + 1) * 128,
                            b * N + ib * 512:b * N + (ib + 1) * 512], osb[:])
                    yield

            def drain(gen):
                for _ in gen:
                    pass

            def chain(*gens):
                for g in gens:
                    yield from g

            def interleave(gen_a, gen_b, na=1, nb=1, prime_b=0):
                """Alternate generators, na steps of gen_a per nb of gen_b,
                optionally priming gen_b first."""
                for _ in range(prime_b):
                    try:
                        next(gen_b)
                    except StopIteration:
                        break
                alive = [gen_a, gen_b]
                while alive:
                    for g in list(alive):
                        steps = na if g is gen_a else nb
                        for _ in range(steps):
                            try:
                                next(g)
                            except StopIteration:
                                if g in alive:
                                    alive.remove(g)
                                break

            # ---- schedule ----
            # Front-load projections (collectives are barrier-gated anyway);
            # run attention (exp-bound) interleaved with remaining q-blocks
            # and output projections; keep the last batch's collectives small.
            xblk_load(0)
            xkv_load(0)
            early_consts()
            xblk_load(1)
            xkv_load(1)
            drain(qkv_block(0))
            xblk_load(2)
            xkv_load(2)
            drain(qkv_block(1))
            kx_exchange(0)
            xblk_load(3)
            xkv_load(3)
            drain(qkv_block(2))
            xblk_load(4)
            drain(qkv_block(3))
            kx_exchange(1)
            xblk_load(5)
            interleave(chain(att_ib(0, 0), att_ib(0, 1)),
                       chain(qkv_block(4), qkv_block(5)), na=1, nb=1, prime_b=4)
            allgather_b(0)
            xblk_load(6)
            xblk_load(7)
            interleave(chain(att_ib(1, 0), att_ib(1, 1)),
                       chain(qkv_block(6), qkv_block(7)), na=1, nb=1, prime_b=2)
            allgather_b(1)
            g_prefetch(0, 0)
            g_prefetch(0, 1)
            interleave(chain(att_ib(2, 0), att_ib(2, 1)),
                       chain(oproj_ib(0, 0), oproj_ib(0, 1)),
                       na=1, nb=1, prime_b=2)
            allgather_b(2)
            g_prefetch(1, 0)
            g_prefetch(1, 1)
            interleave(att_ib(3, 0), chain(oproj_ib(1, 0), oproj_ib(1, 1)),
                       na=1, nb=2, prime_b=2)
            allgather3(0)
            g_prefetch(2, 0)
            g_prefetch(2, 1)
            interleave(att_ib(3, 1), chain(oproj_ib(2, 0), oproj_ib(2, 1)),
                       na=1, nb=2, prime_b=2)
            allgather3(1)
            g_prefetch(3, 0)
            drain(oproj_ib(3, 0))
            g_prefetch(3, 1)
            drain(oproj_ib(3, 1))

    nc.compile()
    _NC_CACHE["nc"] = nc
    return nc


def _host_prep(x, Wq, Wk, Wv, Wo, head_scale):
    bf = ml_dtypes.bfloat16
    xt = np.ascontiguousarray(x.reshape(NT, D).T).astype(bf)

    hs = np.asarray(head_scale).reshape(16)
    wo_s = (np.asarray(Wo) * np.repeat(hs, DH)[:, None]).astype(np.float32)

    def ktile(w):  # [2048, M] -> [128, 16, M]
        m = w.shape[1]
        return np.ascontiguousarray(
            w.reshape(KT, 128, m).transpose(1, 0, 2)).astype(bf)

    inv_freq = (1.0 / (10000.0 ** (np.arange(0, DH, 2, dtype=np.float64) / DH)))
    freqs = np.arange(N, dtype=np.float64)[:, None] * inv_freq[None, :]  # [N, 64]
    emb = np.concatenate([freqs, freqs], axis=-1)  # [N, 128]
    cosT = np.ascontiguousarray(np.cos(emb).T).astype(bf)  # [128, N]
    sinT = np.sin(emb).T  # [128, N]
    sign = np.where(np.arange(DH) < 64, -1.0, 1.0)[:, None]
    sinT = np.ascontiguousarray(sinT * sign).astype(bf)

    # 4 diagonal masks r=0..3: valid (c >= p + 128*r)
    p = np.arange(128)[:, None]
    c = np.arange(512)[None, :]
    masks = [(c >= p + 128 * r).astype(np.float32) for r in range(4)]
    mask = np.concatenate(masks, axis=1).astype(bf)  # [128, 2048]

    idm = np.eye(128, dtype=np.float32).astype(bf)

    in_maps = []
    for core in range(N_CORES):
        kv = core // 2
        half = core % 2
        in_maps.append({
            "xt": xt,
            "xkv": np.ascontiguousarray(xt[:, half * 2048:(half + 1) * 2048]),
            "wq": ktile(np.asarray(Wq)[:, core * 256:(core + 1) * 256]),
            "wk": ktile(np.asarray(Wk)[:, kv * 128:(kv + 1) * 128]),
            "wv": ktile(np.asarray(Wv)[:, kv * 128:(kv + 1) * 128]),
            "wo": ktile(wo_s[:, core * 256:(core + 1) * 256]),
            "cost": cosT,
            "sint": sinT,
            "mask": mask,
            "ident": idm,
        })
    return in_maps


def kernel(x, Wq, Wk, Wv, Wo, head_scale, _run_kwargs=None):
    nc = build_nc()
    in_maps = _host_prep(x, Wq, Wk, Wv, Wo, head_scale)
    res = run_bass_kernel_spmd(
        nc, in_maps, core_ids=list(range(N_CORES)), **(_run_kwargs or {})
    )
    outT = np.concatenate(
        [res.results[c]["out"].astype(np.float32) for c in range(N_CORES)], axis=0)
    full = np.ascontiguousarray(outT.T).reshape(B, N, D)
    if _run_kwargs:
        kernel.last_results = res
    return full


# revision 16
# speedup vs baseline: 1.0591x; 1.0271x over previous
"""Trainium2 8-core kernel for causal GQA attention (nn_Attention_90967407329949).

v1 architecture (no kv dedup) + stall fixes:
 - ag_in writes on gpsimd ring (off the big-load lanes)
 - oproj schedule shifted one segment later (dodges the slow first AllGather)
 - tail reordered: oproj(3,0) interleaved with att(3,1); only oproj(3,1)
   after the last collective
 - startup: wq chunk0 + xblk0 first on their lanes

Distribution: tensor-parallel over query heads (2 q-heads + shared kv-head
per core); per (batch, half) AllGathers of attention outputs; each core
computes a 256-column slice of the output projection.

V tiles are transposed on the PE (identity matmul), NOT dma_start_transpose
(Tile serializes DMA transposes with collectives).
"""

import numpy as np
import ml_dtypes

import concourse.bacc as bacc
import concourse.mybir as mybir
import concourse.tile as tile
from concourse.bass_utils import run_bass_kernel_spmd

BF16 = mybir.dt.bfloat16
F32 = mybir.dt.float32

N_CORES = 8
B = 4
N = 1024
NT = B * N
D = 2048
DH = 128
KT = D // 128
SCALE = 1.0 / np.sqrt(DH)

_NC_CACHE = {}


def build_nc():
    if "nc" in _NC_CACHE:
        return _NC_CACHE["nc"]
    nc = bacc.Bacc("TRN2", target_bir_lowering=False, debug=False, num_devices=N_CORES)

    xt = nc.dram_tensor("xt", [D, NT], BF16, kind="ExternalInput")
    wq = nc.dram_tensor("wq", [128, KT, 256], BF16, kind="ExternalInput")
    wk = nc.dram_tensor("wk", [128, KT, 128], BF16, kind="ExternalInput")
    wv = nc.dram_tensor("wv", [128, KT, 128], BF16, kind="ExternalInput")
    wo = nc.dram_tensor("wo", [128, KT, 256], BF16, kind="ExternalInput")
    cost = nc.dram_tensor("cost", [128, N], BF16, kind="ExternalInput")
    sint = nc.dram_tensor("sint", [128, N], BF16, kind="ExternalInput")
    mask = nc.dram_tensor("mask", [128, 2048], BF16, kind="ExternalInput")
    ident = nc.dram_tensor("ident", [128, 128], BF16, kind="ExternalInput")
    out = nc.dram_tensor("out", [256, NT], BF16, kind="ExternalOutput")

    SEGS = [(b, ib) for b in range(B) for ib in range(2)]
    ag_in = {s: nc.dram_tensor(f"agi{s[0]}{s[1]}", [256, 512], BF16) for s in SEGS}
    ag_out = {s: nc.dram_tensor(f"ago{s[0]}{s[1]}", [D, 512], BF16,
                                addr_space="Shared") for s in SEGS}
    ag_out_r = {s: t.rearrange("(t p) n -> p t n", p=128) for s, t in ag_out.items()}

    with tile.TileContext(nc) as tc:
        with (
            tc.tile_pool(name="const", bufs=1) as constp,
            tc.tile_pool(name="persist", bufs=1) as persist,
            tc.tile_pool(name="xtp", bufs=3) as xtp,
            tc.tile_pool(name="qkraw", bufs=2) as qkrawp,
            tc.tile_pool(name="rope", bufs=2) as ropep,
            tc.tile_pool(name="ep", bufs=4) as ep,
            tc.tile_pool(name="etmpp", bufs=2) as etmpp,
            tc.tile_pool(name="attp", bufs=2) as attp,
            tc.tile_pool(name="recipp", bufs=2) as recipp,
            tc.tile_pool(name="rbcp", bufs=2) as rbcp,
            tc.tile_pool(name="gp", bufs=2) as gp,
            tc.tile_pool(name="oobp", bufs=2) as oobp,
            tc.tile_pool(name="psacc", bufs=3, space="PSUM") as psacc,
            tc.tile_pool(name="pss", bufs=2, space="PSUM") as pss,
            tc.tile_pool(name="psu", bufs=2, space="PSUM") as psu,
            tc.tile_pool(name="pssum", bufs=1, space="PSUM") as pssum,
        ):
            # ---- constants ----
            wq_sb = constp.tile([128, KT, 256], BF16)
            wk_sb = constp.tile([128, KT, 128], BF16)
            wv_sb = constp.tile([128, KT, 128], BF16)
            wo_sb = constp.tile([128, KT, 256], BF16)
            cos_sb = constp.tile([128, N], BF16)
            sin_sb = constp.tile([128, N], BF16)
            mask_sb = constp.tile([128, 2048], BF16)
            ones_sb = constp.tile([128, 1], BF16)
            id_sb = constp.tile([128, 128], BF16)
            nc.scalar.dma_start(wq_sb[:, 0:4, :], wq[:, 0:4, :])
            nc.vector.memset(ones_sb[:], 1.0)

            def early_consts():
                for c in range(1, 4):
                    nc.scalar.dma_start(wq_sb[:, c * 4:(c + 1) * 4, :],
                                        wq[:, c * 4:(c + 1) * 4, :])
                nc.scalar.dma_start(wk_sb[:], wk[:])
                nc.scalar.dma_start(wv_sb[:], wv[:])
                nc.scalar.dma_start(id_sb[:], ident[:])
                nc.scalar.dma_start(cos_sb[:], cost[:])
                nc.scalar.dma_start(sin_sb[:], sint[:])

            def late_consts():
                nc.scalar.dma_start(wo_sb[:], wo[:])
                nc.scalar.dma_start(mask_sb[:], mask[:])

            q_sb = [persist.tile([128, NT], BF16, name=f"q{h}_sb") for h in range(2)]
            k_sb = persist.tile([128, NT], BF16)
            v_sb = persist.tile([128, NT], BF16)

            xt_r = xt.rearrange("(t p) n -> p t n", p=128)

            xblks = {}

            def xblk_load(nb):
                col0 = nb * 512
                xblk = xtp.tile([128, KT, 512], BF16, tag="xblk", name=f"xblk_{nb}")
                ring = nc.sync if nb % 2 == 0 else nc.scalar
                csz = 4 if nb == 0 else 8
                for c0 in range(0, KT, csz):
                    ring.dma_start(xblk[:, c0:c0 + csz, :],
                                   xt_r[:, c0:c0 + csz, col0:col0 + 512])
                xblks[nb] = xblk

            def rope_chunk(raw, dst, c0, col0):
                rot = ropep.tile([128, 512], BF16, tag="rot")
                nc.sync.dma_start(rot[0:64, :], raw[64:128, :])
                nc.sync.dma_start(rot[64:128, :], raw[0:64, :])
                t1 = ropep.tile([128, 512], BF16, tag="t1")
                nc.vector.tensor_mul(t1[:], raw[:], cos_sb[:, c0:c0 + 512])
                t2 = ropep.tile([128, 512], BF16, tag="t2")
                nc.vector.tensor_mul(t2[:], rot[:], sin_sb[:, c0:c0 + 512])
                nc.vector.tensor_add(dst[:, col0:col0 + 512], t1[:], t2[:])

            def qkv_block(nb):
                col0 = nb * 512
                c0 = (nb % 2) * 512
                xblk = xblks[nb]
                if nb == 1:
                    late_consts()

                def accum(dst_ps, w_sb, msl):
                    for k0 in range(0, KT, 4):
                        for kt in range(k0, k0 + 4):
                            nc.tensor.matmul(
                                dst_ps, w_sb[:, kt, msl], xblk[:, kt, :],
                                start=(kt == 0), stop=(kt == KT - 1))
                        yield

                for m in range(2):
                    raw = qkrawp.tile([128, 512], BF16, tag=f"qraw{m}",
                                      name=f"qraw{m}_{nb}")
                    q_ps = psacc.tile([128, 512], F32, tag="psacc",
                                      name=f"q_ps_{nb}_{m}")
                    yield from accum(q_ps[:], wq_sb,
                                     slice(m * 128, (m + 1) * 128))
                    nc.scalar.activation(raw[:], q_ps[:],
                                         mybir.ActivationFunctionType.Copy)
                    yield
                    rope_chunk(raw, q_sb[m], c0, col0)
                kraw = qkrawp.tile([128, 512], BF16, tag="kraw", name=f"kraw_{nb}")
                k_ps = psacc.tile([128, 512], F32, tag="psacc", name=f"k_ps_{nb}")
                yield from accum(k_ps[:], wk_sb, slice(0, 128))
                nc.scalar.activation(kraw[:], k_ps[:],
                                     mybir.ActivationFunctionType.Copy)
                yield
                v_ps = psacc.tile([128, 512], F32, tag="psacc", name=f"v_ps_{nb}")
                yield from accum(v_ps[:], wv_sb, slice(0, 128))
                vraw = ropep.tile([128, 512], BF16, tag="vraw")
                nc.scalar.activation(vraw[:], v_ps[:],
                                     mybir.ActivationFunctionType.Copy)
                yield
                rope_chunk(kraw, k_sb, c0, col0)
                vt_ps = psacc.tile([128, 1024], BF16, tag="psacc",
                                   name=f"vt_ps_{nb}")
                for i in range(4):
                    nc.tensor.matmul(vt_ps[:, i * 128:(i + 1) * 128],
                                     vraw[:, i * 128:(i + 1) * 128], id_sb[:],
                                     is_transpose=True, skip_group_check=True)
                nc.scalar.activation(v_sb[:, col0:col0 + 512], vt_ps[:, 0:512],
                                     mybir.ActivationFunctionType.Copy)
                yield

            def att_ib(b, ib):
                icol = b * N + ib * 512
                cnt = 4 * ib + 4
                for h in range(2):
                    qh = q_sb[h]
                    att = attp.tile([128, 512], BF16, tag="att",
                                    name=f"att_{b}_{ib}_{h}")
                    u_ps = psu.tile([128, 512], F32, tag="psu",
                                    name=f"u_ps_{b}_{ib}_{h}")
                    sum_ps = pssum.tile([1, 512], F32, tag="pssum",
                                        name=f"sum_ps_{b}_{ib}_{h}")

                    def c_lo(jt):
                        r = jt - 4 * ib
                        return 128 * r if r > 0 else 0

                    def s_mm(jt):
                        s_ps = pss.tile([128, 512], F32, tag="pss",
                                        name=f"s_ps_{b}_{ib}_{h}_{jt}")
                        jcol = b * N + jt * 128
                        c0 = c_lo(jt)
                        nc.tensor.matmul(
                            s_ps[:, c0:512], k_sb[:, jcol:jcol + 128],
                            qh[:, icol + c0:icol + 512],
                            start=True, stop=True)
                        return s_ps

                    def e_of(jt, s_ps):
                        r = jt - 4 * ib
                        c0 = c_lo(jt)
                        e = ep.tile([128, 512], BF16, tag="e",
                                    name=f"e_{b}_{ib}_{h}_{jt}")
                        if r >= 0:
                            etmp = etmpp.tile([128, 512], BF16, tag="etmp")
                            nc.scalar.activation(
                                etmp[:, c0:512], s_ps[:, c0:512],
                                mybir.ActivationFunctionType.Exp, scale=SCALE)
                            nc.vector.tensor_mul(
                                e[:, c0:512], etmp[:, c0:512],
                                mask_sb[:, r * 512 + c0:(r + 1) * 512])
                        else:
                            nc.scalar.activation(
                                e[:], s_ps[:],
                                mybir.ActivationFunctionType.Exp, scale=SCALE)
                        return e

                    s_tiles = {0: s_mm(0), 1: s_mm(1)}
                    for jt in range(cnt):
                        e = e_of(jt, s_tiles.pop(jt))
                        if jt + 2 < cnt:
                            s_tiles[jt + 2] = s_mm(jt + 2)
                        tt = b * 8 + jt
                        c0 = c_lo(jt)
                        nc.tensor.matmul(
                            u_ps[:, c0:512],
                            v_sb[:, tt * 128:(tt + 1) * 128], e[:, c0:512],
                            start=(jt == 0), stop=(jt == cnt - 1),
                            skip_group_check=True)
                        nc.tensor.matmul(
                            sum_ps[:, c0:512], ones_sb[:], e[:, c0:512],
                            start=(jt == 0), stop=(jt == cnt - 1),
                            skip_group_check=True)
                        yield
                    recip = recipp.tile([1, 512], F32, tag="recip")
                    nc.vector.reciprocal_approx_fast(out=recip[:], in_=sum_ps[:])
                    rbc = rbcp.tile([128, 512], F32, tag="rbc")
                    nc.gpsimd.partition_broadcast(rbc[:], recip[:])
                    nc.vector.tensor_mul(att[:], u_ps[:], rbc[:])
                    nc.gpsimd.dma_start(
                        ag_in[(b, ib)][h * 128:(h + 1) * 128, :], att[:])
                    yield

            def allgather(b, ib):
                nc.gpsimd.collective_compute(
                    "AllGather",
                    mybir.AluOpType.bypass,
                    replica_groups=[list(range(N_CORES))],
                    ins=[ag_in[(b, ib)][:].opt()],
                    outs=[ag_out[(b, ib)][:].opt()],
                )

            g_tiles = {}

            def g_prefetch(b, ib):
                g_tiles[(b, ib)] = gp.tile([128, KT, 512], BF16, tag="g",
                                           name=f"g_{b}_{ib}")
                for c0 in range(0, KT, 8):
                    nc.gpsimd.dma_start(g_tiles[(b, ib)][:, c0:c0 + 8, :],
                                        ag_out_r[(b, ib)][:, c0:c0 + 8, :])

            def oproj_ib(b, ib):
                g = g_tiles.pop((b, ib))
                for m in range(2):
                    o_ps = psacc.tile([128, 512], F32, tag="psacc",
                                      name=f"o_ps_{b}_{ib}_{m}")
                    for k0 in range(0, KT, 4):
                        for kt in range(k0, k0 + 4):
                            nc.tensor.matmul(
                                o_ps[:], wo_sb[:, kt, m * 128:(m + 1) * 128],
                                g[:, kt, :], start=(kt == 0),
                                stop=(kt == KT - 1))
                        yield
                    osb = oobp.tile([128, 512], BF16, tag="osb",
                                    name=f"osb_{b}_{ib}_{m}")
                    nc.vector.tensor_copy(osb[:], o_ps[:])
                    nc.gpsimd.dma_start(
                        out[m * 128:(m + 1) * 128,
                            b * N + ib * 512:b * N + (ib + 1) * 512], osb[:])
                    yield

            def drain(gen):
                for _ in gen:
                    pass

            def chain(*gens):
                for g in gens:
                    yield from g

            def interleave(gen_a, gen_b, na=1, nb=1, prime_b=0):
                for _ in range(prime_b):
                    try:
                        next(gen_b)
                    except StopIteration:
                        break
                alive = [gen_a, gen_b]
                while alive:
                    for g in list(alive):
                        steps = na if g is gen_a else nb
                        for _ in range(steps):
                            try:
                                next(g)
                            except StopIteration:
                                if g in alive:
                                    alive.remove(g)
                                break

            # ---- schedule ----
            xblk_load(0)
            early_consts()
            xblk_load(1)
            drain(qkv_block(0))
            xblk_load(2)
            interleave(att_ib(0, 0), qkv_block(1), na=1, nb=3, prime_b=4)
            allgather(0, 0)
            xblk_load(3)
            interleave(att_ib(0, 1), qkv_block(2), na=1, nb=1, prime_b=4)
            allgather(0, 1)
            xblk_load(4)
            interleave(att_ib(1, 0), qkv_block(3), na=1, nb=3, prime_b=4)
            allgather(1, 0)
            g_prefetch(0, 0)
            xblk_load(5)
            interleave(att_ib(1, 1), qkv_block(4), na=1, nb=1, prime_b=4)
            allgather(1, 1)
            g_prefetch(0, 1)
            xblk_load(6)
            interleave(att_ib(2, 0), chain(qkv_block(5), oproj_ib(0, 0)),
                       na=1, nb=4, prime_b=4)
            allgather(2, 0)
            g_prefetch(1, 0)
            xblk_load(7)
            interleave(att_ib(2, 1), chain(qkv_block(6), oproj_ib(0, 1)),
                       na=1, nb=2, prime_b=4)
            allgather(2, 1)
            g_prefetch(1, 1)
            interleave(att_ib(3, 0), chain(qkv_block(7), oproj_ib(1, 0)),
                       na=1, nb=4, prime_b=4)
            allgather(3, 0)
            g_prefetch(2, 0)
            g_prefetch(2, 1)
            g_prefetch(3, 0)
            interleave(att_ib(3, 1),
                       chain(oproj_ib(1, 1), oproj_ib(2, 0), oproj_ib(2, 1),
                             oproj_ib(3, 0)),
                       na=1, nb=3, prime_b=2)
            allgather(3, 1)
            g_prefetch(3, 1)
            drain(oproj_ib(3, 1))

    nc.compile()
    _NC_CACHE["nc"] = nc
    return nc


def _host_prep(x, Wq, Wk, Wv, Wo, head_scale):
    bf = ml_dtypes.bfloat16
    xt = np.ascontiguousarray(x.reshape(NT, D).T).astype(bf)

    hs = np.asarray(head_scale).reshape(16)
    wo_s = (np.asarray(Wo) * np.repeat(hs, DH)[:, None]).astype(np.float32)

    def ktile(w):
        m = w.shape[1]
        return np.ascontiguousarray(
            w.reshape(KT, 128, m).transpose(1, 0, 2)).astype(bf)

    inv_freq = (1.0 / (10000.0 ** (np.arange(0, DH, 2, dtype=np.float64) / DH)))
    freqs = np.arange(N, dtype=np.float64)[:, None] * inv_freq[None, :]
    emb = np.concatenate([freqs, freqs], axis=-1)
    cosT = np.ascontiguousarray(np.cos(emb).T).astype(bf)
    sinT = np.sin(emb).T
    sign = np.where(np.arange(DH) < 64, -1.0, 1.0)[:, None]
    sinT = np.ascontiguousarray(sinT * sign).astype(bf)

    p = np.arange(128)[:, None]
    c = np.arange(512)[None, :]
    masks = [(c >= p + 128 * r).astype(np.float32) for r in range(4)]
    mask = np.concatenate(masks, axis=1).astype(bf)

    idm = np.eye(128, dtype=np.float32).astype(bf)

    in_maps = []
    for core in range(N_CORES):
        kv = core // 2
        in_maps.append({
            "xt": xt,
            "wq": ktile(np.asarray(Wq)[:, core * 256:(core + 1) * 256]),
            "wk": ktile(np.asarray(Wk)[:, kv * 128:(kv + 1) * 128]),
            "wv": ktile(np.asarray(Wv)[:, kv * 128:(kv + 1) * 128]),
            "wo": ktile(wo_s[:, core * 256:(core + 1) * 256]),
            "cost": cosT,
            "sint": sinT,
            "mask": mask,
            "ident": idm,
        })
    return in_maps


def kernel(x, Wq, Wk, Wv, Wo, head_scale, _run_kwargs=None):
    nc = build_nc()
    in_maps = _host_prep(x, Wq, Wk, Wv, Wo, head_scale)
    res = run_bass_kernel_spmd(
        nc, in_maps, core_ids=list(range(N_CORES)), **(_run_kwargs or {})
    )
    outT = np.concatenate(
        [res.results[c]["out"].astype(np.float32) for c in range(N_CORES)], axis=0)
    full = np.ascontiguousarray(outT.T).reshape(B, N, D)
    if _run_kwargs:
        kernel.last_results = res
    return full
